# revision 3
# baseline (speedup 1.0000x reference)
"""Trainium2 Bass kernel for nn_CANN_75857712382071.

Single-head self-attention (B=32, A=2048, D=128) with scalar output
projection, algebraically collapsed:

    out[b,aq] = (sum_ak E * (w+c+bo)) / (sum_ak E)
    E = exp(scale * (z M z^T + 1 (x) g)),  M = Wq^T Wk
    g[ak] = z[ak] . (Wk^T bq),   w[ak] = z[ak] . (Wv^T Wo^T)

q/k/v/h are never materialized; the softmax max-subtraction is skipped
(logits are O(10), exactly softmax-invariant in real arithmetic).

Data-parallel over batch: 4 batches per core on 8 NeuronCores.

Per core, per batch:
  1. z tiles [128,128] DMA'd, PE-transposed -> zT (f32r) [d=128, a=2048]
  2. UT = M @ zT + gw (x) 1  (f32r), via PE + DVE bias-add
  3. w column built via 16 N=1 matmuls, packed with ones into wl (bf16)
  4. for each ak-tile: scores sT[ak,aq] = zT_tile^T . UT (f32r, PE),
     exp via ACT (width-1024 ops, bf16 out), then [w|1] reduction
     matmul (bf16) accumulating num/den in PSUM
  5. finale: num/den gathered, reciprocal+mul on DVE, DMA out
"""

import sys
import types

import numpy as np

N_CORES = 8
B, A, D = 32, 2048, 128
B_PER = B // N_CORES
SCALE = float(D) ** -0.5


def _install_axon_shim():
    """Allow run_bass_kernel_spmd(trace=True) to NTFF-profile under axon."""
    try:
        import antenv  # noqa: F401
    except ImportError:
        return
    if "antenv.axon_hooks" not in sys.modules:
        mod = types.ModuleType("antenv.axon_hooks")
        _hook = [None]
        mod.set_axon_ntff_profile_hook = lambda h: _hook.__setitem__(0, h)
        mod.get_axon_ntff_profile_hook = lambda: _hook[0]
        sys.modules["antenv.axon_hooks"] = mod
    from antenv.axon_hooks import (
        get_axon_ntff_profile_hook,
        set_axon_ntff_profile_hook,
    )
    if get_axon_ntff_profile_hook() is None:
        try:
            from trn_agent_boot.trn_boot import _ntff_profile_via_ctypes
            set_axon_ntff_profile_hook(
                _ntff_profile_via_ctypes("/opt/axon/libaxon_pjrt.so"))
        except Exception:
            pass
    try:
        from concourse import bass_utils
        bass_utils.upload_artifacts = lambda tmpdir: tmpdir
    except Exception:
        pass


def _build_program(cbo: float):
    import concourse.bacc as bacc
    import concourse.mybir as mybir
    import concourse.tile as tile
    from concourse import masks

    f32 = mybir.dt.float32
    f32r = mybir.dt.float32r
    bf16 = mybir.dt.bfloat16
    AF = mybir.ActivationFunctionType
    ADD = mybir.AluOpType.add
    MULT = mybir.AluOpType.mult

    nc = bacc.Bacc("TRN2", target_bir_lowering=False, debug=False,
                   num_devices=N_CORES)

    z_d = nc.dram_tensor("z", [B_PER, A, D], f32, kind="ExternalInput").ap()
    m_d = nc.dram_tensor("m_lhs", [D, D], f32, kind="ExternalInput").ap()
    gw_d = nc.dram_tensor("gw", [D, 1], f32, kind="ExternalInput").ap()
    wv_d = nc.dram_tensor("wv", [D, 2], f32, kind="ExternalInput").ap()
    out_d = nc.dram_tensor("out", [B_PER, A], f32, kind="ExternalOutput").ap()

    NT = A // 128          # 16 ak tiles / z tiles
    NH = A // 1024         # 2 aq halves (ACT op width 1024)
    NC_ = A // 512         # 4 aq chunks (nd accumulators)

    with tile.TileContext(nc) as tc:
        with (
            tc.tile_pool(name="sbc", bufs=1) as sbc,
            tc.tile_pool(name="sbz", bufs=4) as sbz,
            tc.tile_pool(name="sbe", bufs=4) as sbe,
            tc.tile_pool(name="sbb", bufs=2) as sbb,
            tc.tile_pool(name="ps_sc", bufs=2, space="PSUM") as ps_sc,
            tc.tile_pool(name="ps_nd", bufs=1, space="PSUM") as ps_nd,
        ):
            # ---- constants ----
            m_f = sbc.tile([D, D], f32)
            nc.sync.dma_start(m_f[:], m_d[:])
            gw_col = sbc.tile([D, 1], f32)
            nc.sync.dma_start(gw_col[:], gw_d[:])
            wv_f = sbc.tile([D, 2], f32)
            nc.sync.dma_start(wv_f[:], wv_d[:])
            ident = sbc.tile([D, D], f32)
            masks.make_identity(nc, ident[:])
            m_r = sbc.tile([D, D], f32r)
            nc.vector.tensor_copy(m_r[:], m_f[:])
            wv_r = sbc.tile([D, 2], f32r)
            nc.vector.tensor_copy(wv_r[:], wv_f[:])

            # ACT table warmup (overlaps first z DMAs)
            warm = sbc.tile([D, 1], f32)
            nc.scalar.activation(warm[:], gw_col[:], AF.Exp, scale=0.0)

            for b in range(B_PER):
                # ---- zT (f32r) via PE transpose ----
                zT = sbb.tile([D, A], f32r, name=f"zT{b}", tag="zT")
                for i in range(NT):
                    zn = sbz.tile([128, 128], f32, name=f"zn{b}_{i}",
                                  tag="zn")
                    nc.sync.dma_start(zn[:], z_d[b, i * 128:(i + 1) * 128, :])
                    pt = ps_sc.tile([128, 128], f32, name=f"pt{b}_{i}",
                                    tag="sc")
                    nc.tensor.transpose(pt[:], zn[:], ident[:])
                    nc.vector.tensor_copy(zT[:, i * 128:(i + 1) * 128], pt[:])

                # ---- UT = M @ zT + gw (x) 1 ----
                UT = sbb.tile([D, A], f32r, name=f"UT{b}", tag="UT")
                for h in range(NH):
                    pu = ps_sc.tile([128, 1024], f32, name=f"pu{b}_{h}",
                                    tag="sc")
                    for j in range(2):
                        o = h * 1024 + j * 512
                        nc.tensor.matmul(pu[:, j * 512:(j + 1) * 512],
                                         m_r[:], zT[:, o:o + 512],
                                         start=True, stop=True)
                    nc.vector.tensor_scalar(
                        UT[:, h * 1024:(h + 1) * 1024], pu[:], gw_col[:],
                        None, ADD)

                # ---- wl = interleave(w + c + bo, ones) in bf16 ----
                pw = ps_sc.tile([128, 2 * NT], f32, name=f"pw{b}", tag="sc")
                for t in range(NT):
                    nc.tensor.matmul(pw[:, 2 * t:2 * t + 2],
                                     zT[:, t * 128:(t + 1) * 128], wv_r[:],
                                     start=True, stop=True)
                wl = sbb.tile([128, 2 * NT], bf16, name=f"wl{b}", tag="wl")
                nc.gpsimd.memset(wl[:], 1.0)
                wl3 = wl.rearrange("p (t two) -> p t two", two=2)
                pw3 = pw.rearrange("p (t two) -> p t two", two=2)
                nc.vector.tensor_scalar(wl3[:, :, 0], pw3[:, :, 0], cbo,
                                        None, ADD)

                # ---- nd accumulators ----
                nd = [ps_nd.tile([2, 512], f32, name=f"nd{b}_{c}",
                                 tag=f"nd{c}") for c in range(NC_)]

                # ---- main loop over ak tiles ----
                for tk in range(NT):
                    lhs = zT[:, tk * 128:(tk + 1) * 128]
                    wlt = wl[:, 2 * tk:2 * tk + 2]
                    for h in range(NH):
                        ps_t = ps_sc.tile([128, 1024], f32,
                                          name=f"s{b}_{tk}_{h}", tag="sc")
                        for j in range(2):
                            o = h * 1024 + j * 512
                            nc.tensor.matmul(ps_t[:, j * 512:(j + 1) * 512],
                                             lhs, UT[:, o:o + 512],
                                             start=True, stop=True)
                        eT = sbe.tile([128, 1024], bf16,
                                      name=f"e{b}_{tk}_{h}", tag="eT")
                        nc.scalar.activation(eT[:], ps_t[:], AF.Exp,
                                             scale=SCALE)
                        for j in range(2):
                            c = 2 * h + j
                            nc.tensor.matmul(
                                nd[c][:], wlt, eT[:, j * 512:(j + 1) * 512],
                                start=(tk == 0), stop=(tk == NT - 1))

                # ---- finale: out = num / den ----
                ndall = sbb.tile([2, A], f32, name=f"ndall{b}", tag="ndall")
                for c in range(NC_):
                    nc.vector.tensor_copy(
                        ndall[0:2, c * 512:(c + 1) * 512], nd[c][:])
                den = sbb.tile([1, A], f32, name=f"den{b}", tag="den")
                nc.sync.dma_start(den[:], ndall[1:2, :])
                rcp = sbb.tile([1, A], f32, name=f"rcp{b}", tag="rcp")
                nc.vector.reciprocal(rcp[:], den[:])
                orow = sbb.tile([1, A], f32, name=f"orow{b}", tag="orow")
                nc.vector.tensor_tensor(orow[:], ndall[0:1, :], rcp[:], MULT)
                nc.sync.dma_start(out_d[b:b + 1, :], orow[:])

    nc.compile()
    return nc


def run(inputs: dict, trace: bool = False):
    _install_axon_shim()
    from concourse.bass_utils import run_bass_kernel_spmd

    z = np.asarray(inputs["z"], dtype=np.float32)
    Wq = np.asarray(inputs["Wq"], dtype=np.float64)
    bq = np.asarray(inputs["bq"], dtype=np.float64)
    Wk = np.asarray(inputs["Wk"], dtype=np.float64)
    Wv = np.asarray(inputs["Wv"], dtype=np.float64)
    bv = np.asarray(inputs["bv"], dtype=np.float64)
    Wo = np.asarray(inputs["Wo"], dtype=np.float64)
    bo = np.asarray(inputs["bo"], dtype=np.float64)

    # host-side weight algebra (tiny, exact in float64)
    m_lhs = (Wq.T @ Wk).astype(np.float32)            # [d, d']
    gw = (Wk.T @ bq).astype(np.float32).reshape(D, 1)
    wv = np.repeat((Wv.T @ Wo[0]).astype(np.float32).reshape(D, 1), 2, axis=1)
    cbo = float(bv @ Wo[0] + bo[0])

    nc = _build_program(cbo)

    in_maps = []
    for c in range(N_CORES):
        in_maps.append({
            "z": z[c * B_PER:(c + 1) * B_PER],
            "m_lhs": m_lhs,
            "gw": gw,
            "wv": wv,
        })
    res = run_bass_kernel_spmd(nc, in_maps, core_ids=list(range(N_CORES)),
                               trace=trace)
    out = np.concatenate([res.results[c]["out"] for c in range(N_CORES)],
                         axis=0)
    return out.reshape(B, A, 1).astype(np.float32), res


def kernel(**inputs) -> np.ndarray:
    out, _ = run(inputs, trace=False)
    return out


# revision 4
# speedup vs baseline: 1.2352x; 1.2352x over previous
"""Trainium2 Bass kernel for nn_CANN_75857712382071.

Single-head self-attention (B=32, A=2048, D=128) with scalar output
projection, algebraically collapsed:

    out[b,aq] = (sum_ak E * (w+c+bo)) / (sum_ak E)
    E = exp(scale * (z M z^T + 1 (x) g)),  M = Wq^T Wk
    g[ak] = z[ak] . (Wk^T bq),   w[ak] = z[ak] . (Wv^T Wo^T)

q/k/v/h are never materialized; softmax max-subtraction is skipped
(logits are O(10); softmax is shift-invariant in exact arithmetic).

Data-parallel over batch: 4 batches per core on 8 NeuronCores.
Batches are software-pipelined: batch b+1's setup (z DMA, PE
transposes to zT, UT = M zT + gw, w column) is emitted in small pieces
inside batch b's main loop so ScalarE (exp) never starves.
"""

import sys
import types

import numpy as np

N_CORES = 8
B, A, D = 32, 2048, 128
B_PER = B // N_CORES
SCALE = float(D) ** -0.5


def _install_axon_shim():
    """Allow run_bass_kernel_spmd(trace=True) to NTFF-profile under axon."""
    try:
        import antenv  # noqa: F401
    except ImportError:
        return
    if "antenv.axon_hooks" not in sys.modules:
        mod = types.ModuleType("antenv.axon_hooks")
        _hook = [None]
        mod.set_axon_ntff_profile_hook = lambda h: _hook.__setitem__(0, h)
        mod.get_axon_ntff_profile_hook = lambda: _hook[0]
        sys.modules["antenv.axon_hooks"] = mod
    from antenv.axon_hooks import (
        get_axon_ntff_profile_hook,
        set_axon_ntff_profile_hook,
    )
    if get_axon_ntff_profile_hook() is None:
        try:
            from trn_agent_boot.trn_boot import _ntff_profile_via_ctypes
            set_axon_ntff_profile_hook(
                _ntff_profile_via_ctypes("/opt/axon/libaxon_pjrt.so"))
        except Exception:
            pass
    try:
        from concourse import bass_utils
        bass_utils.upload_artifacts = lambda tmpdir: tmpdir
    except Exception:
        pass


def _build_program(cbo: float):
    import concourse.bacc as bacc
    import concourse.mybir as mybir
    import concourse.tile as tile
    from concourse import masks

    f32 = mybir.dt.float32
    f32r = mybir.dt.float32r
    bf16 = mybir.dt.bfloat16
    AF = mybir.ActivationFunctionType
    ADD = mybir.AluOpType.add
    MULT = mybir.AluOpType.mult

    nc = bacc.Bacc("TRN2", target_bir_lowering=False, debug=False,
                   num_devices=N_CORES)

    z_d = nc.dram_tensor("z", [B_PER, A, D], f32, kind="ExternalInput").ap()
    m_d = nc.dram_tensor("m_lhs", [D, D], f32, kind="ExternalInput").ap()
    gw_d = nc.dram_tensor("gw", [D, 1], f32, kind="ExternalInput").ap()
    wv_d = nc.dram_tensor("wv", [D, 2], f32, kind="ExternalInput").ap()
    out_d = nc.dram_tensor("out", [B_PER, A], f32, kind="ExternalOutput").ap()

    NT = A // 128          # 16 ak tiles / z tiles
    NH = A // 1024         # 2 aq halves (ACT op width 1024)
    NC_ = A // 512         # 4 aq chunks (nd accumulators)

    with tile.TileContext(nc) as tc:
        with (
            tc.tile_pool(name="sbc", bufs=1) as sbc,
            tc.tile_pool(name="sbz", bufs=18) as sbz,
            tc.tile_pool(name="sbe", bufs=6) as sbe,
            tc.tile_pool(name="sbb", bufs=2) as sbb,
            tc.tile_pool(name="ps_sc", bufs=2, space="PSUM") as ps_sc,
            tc.tile_pool(name="ps_nd", bufs=1, space="PSUM") as ps_nd,
        ):
            # ---- constants ----
            m_f = sbc.tile([D, D], f32)
            nc.sync.dma_start(m_f[:], m_d[:])
            gw_col = sbc.tile([D, 1], f32)
            nc.sync.dma_start(gw_col[:], gw_d[:])
            wv_f = sbc.tile([D, 2], f32)
            nc.sync.dma_start(wv_f[:], wv_d[:])
            ident = sbc.tile([D, D], f32)
            masks.make_identity(nc, ident[:])
            m_r = sbc.tile([D, D], f32r)
            nc.vector.tensor_copy(m_r[:], m_f[:])
            wv_r = sbc.tile([D, 2], f32r)
            nc.vector.tensor_copy(wv_r[:], wv_f[:])

            # ACT table warmup (overlaps first z DMAs)
            warm = sbc.tile([D, 1], f32)
            nc.scalar.activation(warm[:], gw_col[:], AF.Exp, scale=0.0)

            st = {}  # per-batch live tiles

            def emit_z_dmas(b):
                s = st.setdefault(b, {})
                s["zn"] = []
                s["zT"] = sbb.tile([D, A], f32r, name=f"zT{b}", tag="zT")
                for i in range(NT):
                    zn = sbz.tile([128, 128], f32, name=f"zn{b}_{i}",
                                  tag="zn")
                    nc.sync.dma_start(zn[:], z_d[b, i * 128:(i + 1) * 128, :])
                    s["zn"].append(zn)

            def emit_transpose_piece(b, lo, hi):
                s = st[b]
                for i in range(lo, hi):
                    pt = ps_sc.tile([128, 128], f32, name=f"pt{b}_{i}",
                                    tag="sc")
                    nc.tensor.transpose(pt[:], s["zn"][i][:], ident[:])
                    nc.vector.tensor_copy(
                        s["zT"][:, i * 128:(i + 1) * 128], pt[:])

            def emit_ut_w(b):
                s = st[b]
                zT = s["zT"]
                UT = sbb.tile([D, A], f32r, name=f"UT{b}", tag="UT")
                for h in range(NH):
                    pu = ps_sc.tile([128, 1024], f32, name=f"pu{b}_{h}",
                                    tag="sc")
                    for j in range(2):
                        o = h * 1024 + j * 512
                        nc.tensor.matmul(pu[:, j * 512:(j + 1) * 512],
                                         m_r[:], zT[:, o:o + 512],
                                         start=True, stop=True)
                    nc.vector.tensor_scalar(
                        UT[:, h * 1024:(h + 1) * 1024], pu[:], gw_col[:],
                        None, ADD)
                s["UT"] = UT
                pw = ps_sc.tile([128, 2 * NT], f32, name=f"pw{b}", tag="sc")
                for t in range(NT):
                    nc.tensor.matmul(pw[:, 2 * t:2 * t + 2],
                                     zT[:, t * 128:(t + 1) * 128], wv_r[:],
                                     start=True, stop=True)
                wl = sbb.tile([128, 2 * NT], bf16, name=f"wl{b}", tag="wl")
                nc.gpsimd.memset(wl[:], 1.0)
                wl3 = wl.rearrange("p (t two) -> p t two", two=2)
                pw3 = pw.rearrange("p (t two) -> p t two", two=2)
                nc.vector.tensor_scalar(wl3[:, :, 0], pw3[:, :, 0], cbo,
                                        None, ADD)
                s["wl"] = wl

            def emit_main_tk(b, tk):
                s = st[b]
                lhs = s["zT"][:, tk * 128:(tk + 1) * 128]
                wlt = s["wl"][:, 2 * tk:2 * tk + 2]
                for h in range(NH):
                    ps_t = ps_sc.tile([128, 1024], f32,
                                      name=f"s{b}_{tk}_{h}", tag="sc")
                    for j in range(2):
                        o = h * 1024 + j * 512
                        nc.tensor.matmul(ps_t[:, j * 512:(j + 1) * 512],
                                         lhs, s["UT"][:, o:o + 512],
                                         start=True, stop=True)
                    eT = sbe.tile([128, 1024], bf16,
                                  name=f"e{b}_{tk}_{h}", tag="eT")
                    nc.scalar.activation(eT[:], ps_t[:], AF.Exp, scale=SCALE)
                    for j in range(2):
                        c = 2 * h + j
                        nc.tensor.matmul(
                            s["nd"][c][:], wlt,
                            eT[:, j * 512:(j + 1) * 512],
                            start=(tk == 0), stop=(tk == NT - 1))

            def emit_finale(b):
                s = st[b]
                ndall = sbb.tile([2, A], f32, name=f"ndall{b}", tag="ndall")
                for c in range(NC_):
                    nc.vector.tensor_copy(
                        ndall[0:2, c * 512:(c + 1) * 512], s["nd"][c][:])
                den = sbb.tile([1, A], f32, name=f"den{b}", tag="den")
                nc.sync.dma_start(den[:], ndall[1:2, :])
                rcp = sbb.tile([1, A], f32, name=f"rcp{b}", tag="rcp")
                nc.vector.reciprocal(rcp[:], den[:])
                orow = sbb.tile([1, A], f32, name=f"orow{b}", tag="orow")
                nc.vector.tensor_tensor(orow[:], ndall[0:1, :], rcp[:], MULT)
                nc.sync.dma_start(out_d[b:b + 1, :], orow[:])
                st.pop(b)

            # ---- prologue: batch 0 setup ----
            emit_z_dmas(0)
            for p in range(4):
                emit_transpose_piece(0, 4 * p, 4 * p + 4)
            emit_ut_w(0)

            for b in range(B_PER):
                s = st[b]
                s["nd"] = [ps_nd.tile([2, 512], f32, name=f"nd{b}_{c}",
                                      tag=f"nd{c}") for c in range(NC_)]
                nxt = b + 1 if b + 1 < B_PER else None
                for tk in range(NT):
                    emit_main_tk(b, tk)
                    if nxt is not None:
                        if tk == 4:
                            emit_z_dmas(nxt)
                        elif tk in (6, 8, 10, 12):
                            p = (tk - 6) // 2
                            emit_transpose_piece(nxt, 4 * p, 4 * p + 4)
                        elif tk == 14:
                            emit_ut_w(nxt)
                emit_finale(b)

    nc.compile()
    return nc


def run(inputs: dict, trace: bool = False):
    _install_axon_shim()
    from concourse.bass_utils import run_bass_kernel_spmd

    z = np.asarray(inputs["z"], dtype=np.float32)
    Wq = np.asarray(inputs["Wq"], dtype=np.float64)
    bq = np.asarray(inputs["bq"], dtype=np.float64)
    Wk = np.asarray(inputs["Wk"], dtype=np.float64)
    Wv = np.asarray(inputs["Wv"], dtype=np.float64)
    bv = np.asarray(inputs["bv"], dtype=np.float64)
    Wo = np.asarray(inputs["Wo"], dtype=np.float64)
    bo = np.asarray(inputs["bo"], dtype=np.float64)

    # host-side weight algebra (tiny, exact in float64)
    m_lhs = (Wq.T @ Wk).astype(np.float32)            # [d, d']
    gw = (Wk.T @ bq).astype(np.float32).reshape(D, 1)
    wv = np.repeat((Wv.T @ Wo[0]).astype(np.float32).reshape(D, 1), 2, axis=1)
    cbo = float(bv @ Wo[0] + bo[0])

    nc = _build_program(cbo)

    in_maps = []
    for c in range(N_CORES):
        in_maps.append({
            "z": z[c * B_PER:(c + 1) * B_PER],
            "m_lhs": m_lhs,
            "gw": gw,
            "wv": wv,
        })
    res = run_bass_kernel_spmd(nc, in_maps, core_ids=list(range(N_CORES)),
                               trace=trace)
    out = np.concatenate([res.results[c]["out"] for c in range(N_CORES)],
                         axis=0)
    return out.reshape(B, A, 1).astype(np.float32), res


def kernel(**inputs) -> np.ndarray:
    out, _ = run(inputs, trace=False)
    return out


# revision 6
# speedup vs baseline: 1.3583x; 1.0996x over previous
"""Trainium2 Bass kernel for nn_CANN_75857712382071.

Single-head self-attention (B=32, A=2048, D=128) with scalar output
projection, algebraically collapsed:

    out[b,aq] = (sum_ak E * (w+c+bo)) / (sum_ak E)
    E = exp(scale * (z M z^T + 1 (x) g)),  M = Wq^T Wk
    g[ak] = z[ak] . (Wk^T bq),   w[ak] = z[ak] . (Wv^T Wo^T)

q/k/v/h are never materialized; softmax max-subtraction is skipped
(logits are O(10); softmax is shift-invariant in exact arithmetic).

Data-parallel over batch: 4 batches per core on 8 NeuronCores.
Batches are software-pipelined: batch b+1's setup (z DMA, PE
transposes to zT, UT = M zT + gw, w column) is emitted in small pieces
inside batch b's main loop so ScalarE (exp) never starves.
"""

import sys
import types

import numpy as np

N_CORES = 8
B, A, D = 32, 2048, 128
B_PER = B // N_CORES
SCALE = float(D) ** -0.5


def _install_axon_shim():
    """Allow run_bass_kernel_spmd(trace=True) to NTFF-profile under axon."""
    try:
        import antenv  # noqa: F401
    except ImportError:
        return
    if "antenv.axon_hooks" not in sys.modules:
        mod = types.ModuleType("antenv.axon_hooks")
        _hook = [None]
        mod.set_axon_ntff_profile_hook = lambda h: _hook.__setitem__(0, h)
        mod.get_axon_ntff_profile_hook = lambda: _hook[0]
        sys.modules["antenv.axon_hooks"] = mod
    from antenv.axon_hooks import (
        get_axon_ntff_profile_hook,
        set_axon_ntff_profile_hook,
    )
    if get_axon_ntff_profile_hook() is None:
        try:
            from trn_agent_boot.trn_boot import _ntff_profile_via_ctypes
            set_axon_ntff_profile_hook(
                _ntff_profile_via_ctypes("/opt/axon/libaxon_pjrt.so"))
        except Exception:
            pass
    try:
        from concourse import bass_utils
        bass_utils.upload_artifacts = lambda tmpdir: tmpdir
    except Exception:
        pass


def _build_program(cbo: float):
    import concourse.bacc as bacc
    import concourse.mybir as mybir
    import concourse.tile as tile
    from concourse import masks

    f32 = mybir.dt.float32
    f32r = mybir.dt.float32r
    bf16 = mybir.dt.bfloat16
    AF = mybir.ActivationFunctionType
    ADD = mybir.AluOpType.add
    MULT = mybir.AluOpType.mult

    nc = bacc.Bacc("TRN2", target_bir_lowering=False, debug=False,
                   num_devices=N_CORES)

    z_d = nc.dram_tensor("z", [B_PER, A, D], f32, kind="ExternalInput").ap()
    m_d = nc.dram_tensor("m_lhs", [D, D], f32, kind="ExternalInput").ap()
    gw_d = nc.dram_tensor("gw", [D, 1], f32, kind="ExternalInput").ap()
    wv_d = nc.dram_tensor("wv", [D, 2], f32, kind="ExternalInput").ap()
    out_d = nc.dram_tensor("out", [B_PER, A], f32, kind="ExternalOutput").ap()

    NT = A // 128          # 16 ak tiles / z tiles
    NH = A // 1024         # 2 aq halves (ACT op width 1024)
    NC_ = A // 512         # 4 aq chunks (nd accumulators)

    with tile.TileContext(nc) as tc:
        with (
            tc.tile_pool(name="sbc", bufs=1) as sbc,
            tc.tile_pool(name="sbz", bufs=2) as sbz,
            tc.tile_pool(name="sbe", bufs=6) as sbe,
            tc.tile_pool(name="sbb", bufs=2) as sbb,
            tc.tile_pool(name="ps_sc", bufs=2, space="PSUM") as ps_sc,
            tc.tile_pool(name="ps_nd", bufs=1, space="PSUM") as ps_nd,
        ):
            # ---- constants ----
            m_f = sbc.tile([D, D], f32)
            nc.sync.dma_start(m_f[:], m_d[:])
            gw_col = sbc.tile([D, 1], f32)
            nc.sync.dma_start(gw_col[:], gw_d[:])
            wv_f = sbc.tile([D, 2], f32)
            nc.sync.dma_start(wv_f[:], wv_d[:])
            ident = sbc.tile([D, D], f32)
            masks.make_identity(nc, ident[:])
            m_r = sbc.tile([D, D], f32r)
            nc.vector.tensor_copy(m_r[:], m_f[:])
            wv_r = sbc.tile([D, 2], f32r)
            nc.vector.tensor_copy(wv_r[:], wv_f[:])

            # ACT table warmup (overlaps first z DMAs)
            warm = sbc.tile([D, 1], f32)
            nc.scalar.activation(warm[:], gw_col[:], AF.Exp, scale=0.0)

            st = {}  # per-batch live tiles

            def emit_z_dmas(b):
                s = st.setdefault(b, {})
                s["zT"] = sbb.tile([D, A], f32r, name=f"zT{b}", tag="zT")
                zn = sbz.tile([128, A], f32, name=f"zn{b}", tag="zn")
                zsrc = z_d[b].rearrange("(t p) d -> p t d", p=128)
                nc.sync.dma_start(zn.rearrange("p (t d) -> p t d", d=D), zsrc)
                s["zn"] = zn

            def emit_transpose_group(b, g):
                # 8 transposes into one 2-bank psum slot, then 1 wide copy
                s = st[b]
                pt = ps_sc.tile([128, 1024], f32, name=f"pt{b}_{g}", tag="sc")
                for j in range(8):
                    i = 8 * g + j
                    nc.tensor.transpose(pt[:, j * 128:(j + 1) * 128],
                                        s["zn"][:, i * 128:(i + 1) * 128],
                                        ident[:])
                nc.vector.tensor_copy(
                    s["zT"][:, g * 1024:(g + 1) * 1024], pt[:])

            def emit_ut_w(b):
                s = st[b]
                zT = s["zT"]
                UT = sbb.tile([D, A], f32r, name=f"UT{b}", tag="UT")
                for h in range(NH):
                    pu = ps_sc.tile([128, 1024], f32, name=f"pu{b}_{h}",
                                    tag="sc")
                    for j in range(2):
                        o = h * 1024 + j * 512
                        nc.tensor.matmul(pu[:, j * 512:(j + 1) * 512],
                                         m_r[:], zT[:, o:o + 512],
                                         start=True, stop=True)
                    nc.vector.tensor_scalar(
                        UT[:, h * 1024:(h + 1) * 1024], pu[:], gw_col[:],
                        None, ADD)
                s["UT"] = UT
                pw = ps_sc.tile([128, 2 * NT], f32, name=f"pw{b}", tag="sc")
                for t in range(NT):
                    nc.tensor.matmul(pw[:, 2 * t:2 * t + 2],
                                     zT[:, t * 128:(t + 1) * 128], wv_r[:],
                                     start=True, stop=True)
                wl = sbb.tile([128, 2 * NT], bf16, name=f"wl{b}", tag="wl")
                nc.gpsimd.memset(wl[:], 1.0)
                wl3 = wl.rearrange("p (t two) -> p t two", two=2)
                pw3 = pw.rearrange("p (t two) -> p t two", two=2)
                nc.vector.tensor_scalar(wl3[:, :, 0], pw3[:, :, 0], cbo,
                                        None, ADD)
                s["wl"] = wl

            def emit_main_tk(b, tk):
                s = st[b]
                lhs = s["zT"][:, tk * 128:(tk + 1) * 128]
                wlt = s["wl"][:, 2 * tk:2 * tk + 2]
                for h in range(NH):
                    ps_t = ps_sc.tile([128, 1024], f32,
                                      name=f"s{b}_{tk}_{h}", tag="sc")
                    for j in range(2):
                        o = h * 1024 + j * 512
                        nc.tensor.matmul(ps_t[:, j * 512:(j + 1) * 512],
                                         lhs, s["UT"][:, o:o + 512],
                                         start=True, stop=True)
                    eT = sbe.tile([128, 1024], bf16,
                                  name=f"e{b}_{tk}_{h}", tag="eT")
                    nc.scalar.activation(eT[:], ps_t[:], AF.Exp, scale=SCALE)
                    for j in range(2):
                        c = 2 * h + j
                        nc.tensor.matmul(
                            s["nd"][c][:], wlt,
                            eT[:, j * 512:(j + 1) * 512],
                            start=(tk == 0), stop=(tk == NT - 1))

            def emit_finale(b):
                s = st[b]
                ndall = sbb.tile([2, A], f32, name=f"ndall{b}", tag="ndall")
                for c in range(NC_):
                    nc.vector.tensor_copy(
                        ndall[0:2, c * 512:(c + 1) * 512], s["nd"][c][:])
                den = sbb.tile([1, A], f32, name=f"den{b}", tag="den")
                nc.sync.dma_start(den[:], ndall[1:2, :])
                rcp = sbb.tile([1, A], f32, name=f"rcp{b}", tag="rcp")
                nc.vector.reciprocal(rcp[:], den[:])
                orow = sbb.tile([1, A], f32, name=f"orow{b}", tag="orow")
                nc.vector.tensor_tensor(orow[:], ndall[0:1, :], rcp[:], MULT)
                nc.sync.dma_start(out_d[b:b + 1, :], orow[:])
                st.pop(b)

            # ---- prologue: batch 0 setup ----
            emit_z_dmas(0)
            for g in range(2):
                emit_transpose_group(0, g)
            emit_ut_w(0)

            for b in range(B_PER):
                s = st[b]
                s["nd"] = [ps_nd.tile([2, 512], f32, name=f"nd{b}_{c}",
                                      tag=f"nd{c}") for c in range(NC_)]
                nxt = b + 1 if b + 1 < B_PER else None
                for tk in range(NT):
                    emit_main_tk(b, tk)
                    if nxt is not None:
                        if tk == 4:
                            emit_z_dmas(nxt)
                        elif tk == 7:
                            emit_transpose_group(nxt, 0)
                        elif tk == 10:
                            emit_transpose_group(nxt, 1)
                        elif tk == 13:
                            emit_ut_w(nxt)
                emit_finale(b)

    nc.compile()
    return nc


def run(inputs: dict, trace: bool = False):
    _install_axon_shim()
    from concourse.bass_utils import run_bass_kernel_spmd

    z = np.asarray(inputs["z"], dtype=np.float32)
    Wq = np.asarray(inputs["Wq"], dtype=np.float64)
    bq = np.asarray(inputs["bq"], dtype=np.float64)
    Wk = np.asarray(inputs["Wk"], dtype=np.float64)
    Wv = np.asarray(inputs["Wv"], dtype=np.float64)
    bv = np.asarray(inputs["bv"], dtype=np.float64)
    Wo = np.asarray(inputs["Wo"], dtype=np.float64)
    bo = np.asarray(inputs["bo"], dtype=np.float64)

    # host-side weight algebra (tiny, exact in float64)
    m_lhs = (Wq.T @ Wk).astype(np.float32)            # [d, d']
    gw = (Wk.T @ bq).astype(np.float32).reshape(D, 1)
    wv = np.repeat((Wv.T @ Wo[0]).astype(np.float32).reshape(D, 1), 2, axis=1)
    cbo = float(bv @ Wo[0] + bo[0])

    nc = _build_program(cbo)

    in_maps = []
    for c in range(N_CORES):
        in_maps.append({
            "z": z[c * B_PER:(c + 1) * B_PER],
            "m_lhs": m_lhs,
            "gw": gw,
            "wv": wv,
        })
    res = run_bass_kernel_spmd(nc, in_maps, core_ids=list(range(N_CORES)),
                               trace=trace)
    out = np.concatenate([res.results[c]["out"] for c in range(N_CORES)],
                         axis=0)
    return out.reshape(B, A, 1).astype(np.float32), res


def kernel(**inputs) -> np.ndarray:
    out, _ = run(inputs, trace=False)
    return out
